# revision 3
# baseline (speedup 1.0000x reference)
"""Trainium2 Bass kernel for fused QKV projection + interleaved RoPE.

Problem: X[4, 4096, 2048] @ {Wq, Wk, Wv}[2048, 2048] -> reshape to heads
[B, S, 16, 128], apply interleaved RoPE to Q and K, return (Xq, Xk, Xv).

Sharding: data-parallel over tokens. The 4*4096 = 16384 token rows are
split into 8 contiguous shards of 2048 rows (core c gets batch c//2,
sequence half c%2). Every core holds the full Wq/Wk/Wv and computes all
2048 output features for its rows; RoPE is per-token elementwise so no
communication is needed.

Device kernel (identical SPMD program on all 8 cores):
  - The PE is the bottleneck (3 x [2048,2048]@[2048,2048] bf16 per core).
    A self-loading matmul pays a ~46 ns weight-swap bubble per
    instruction (measured 259 ns for 512 moving rows vs 213 ns
    streaming). So the stationary X^T tile [128k, 128r] is loaded ONCE
    via an explicit LDWEIGHTS and reused by 4 consecutive matmuls
    (ins.ldweights = False) covering all 2048 output columns of one
    tensor in 4 x 512-column PSUM chunks.
  - Per phase (V, Q, K): for rc: 4 psum chunks [128,512] accumulate over
    ko=16 LD+4MM groups. 8 one-bank psum tiles double-buffer rc vs rc+1.
    The first two rc of each phase are interleaved (8 chunks live) so
    the cold W stream is consumed at half rate.
  - RoPE in 3 DVE ops per chunk on the psum tile: the interleaved pair
    swap is a reversed-stride access pattern, the rotation sign is
    pre-baked into the sin table on the host, and cos/sin broadcast
    across heads via zero-stride APs. V chunks are copied on the Act
    engine. Outputs are written f16 (halves DMA-out) and upcast on host.
"""

import numpy as np
import ml_dtypes

import concourse.bass as bass
import concourse.mybir as mybir
import concourse.tile as tile
from concourse import bacc
from concourse.bass import ds, ts
from concourse.bass_utils import run_bass_kernel_spmd

B, S, DIM, H = 4, 4096, 2048, 16
HD = DIM // H           # 128
N_CORES = 8
R = B * S // N_CORES    # 2048 token rows per core
P = 128

BF16 = mybir.dt.bfloat16
F16 = mybir.dt.float16
F32 = mybir.dt.float32

MC = 512                # psum chunk columns (1 PSUM bank)


def build_nc(K=DIM, M=DIM, rows=R, hd=HD):
    """Build the per-core Bass program."""
    KO = K // P           # 16 k-chunks
    RC = rows // P        # 16 token row chunks
    NC_ = M // MC         # 4 column chunks per tensor
    WT = 1024             # W sbuf tile columns
    NH = MC // hd         # heads per chunk (4)
    J = hd // 2

    nc = bacc.Bacc(None, target_bir_lowering=False)

    xt = nc.dram_tensor("xt", [K, rows], BF16, kind="ExternalInput")
    wq = nc.dram_tensor("wq", [K, M], BF16, kind="ExternalInput")
    wk = nc.dram_tensor("wk", [K, M], BF16, kind="ExternalInput")
    wv = nc.dram_tensor("wv", [K, M], BF16, kind="ExternalInput")
    cosf = nc.dram_tensor("cosf", [rows, hd], F32, kind="ExternalInput")
    ssin = nc.dram_tensor("ssin", [rows, hd], F32, kind="ExternalInput")
    q_out = nc.dram_tensor("q", [rows, M], F16, kind="ExternalOutput")
    k_out = nc.dram_tensor("k", [rows, M], F16, kind="ExternalOutput")
    v_out = nc.dram_tensor("v", [rows, M], F16, kind="ExternalOutput")

    xt_r = xt[:].rearrange("(ko p) r -> p ko r", p=P)
    cos_r = cosf[:].rearrange("(rc p) d -> p rc d", p=P)
    sin_r = ssin[:].rearrange("(rc p) d -> p rc d", p=P)

    with tile.TileContext(nc) as tc:
        with (
            tc.tile_pool(name="wpool", bufs=48) as wpool,
            tc.tile_pool(name="xpool", bufs=RC) as xpool,
            tc.tile_pool(name="cpool", bufs=1) as cpool,
            tc.tile_pool(name="opool", bufs=12) as opool,
            tc.tile_pool(name="tpool", bufs=4) as tpool,
            tc.tile_pool(name="psum", bufs=8, space="PSUM") as pspool,
        ):
            def emit_group(xt_tiles, rc, ko, w_tiles, ps_chunks, first, last):
                # one stationary load feeding NC_ matmuls
                nc.tensor.ldweights(xt_tiles[rc][:, ko])
                for c in range(NC_):
                    w_sb = w_tiles[2 * ko + c // 2]
                    mm = nc.tensor.matmul(
                        ps_chunks[c][:],
                        xt_tiles[rc][:, ko],
                        w_sb[:, ts(c % 2, MC)],
                        start=first,
                        stop=last,
                    )
                    mm.ins.ldweights = False

            def finish_chunk(ps, o_r, rc, c, rope, cos_sb, sin_sb):
                o_sb = opool.tile([P, MC], F16, tag="o")
                if rope:
                    # o = x*cos + swap_pairs(x)*ssin; ssin sign-baked,
                    # the swap is a reversed-stride AP on the pair dim.
                    ps_hd = ps[:].rearrange("p (h d) -> p h d", d=hd)
                    ps_pr = ps[:].rearrange(
                        "p (h j two) -> p h j two", h=NH, two=2
                    )
                    cos_b = cos_sb[:, rc, None, :].to_broadcast([P, NH, hd])
                    sin_b = sin_sb[:, rc].rearrange(
                        "p (j two) -> p j two", two=2
                    )[:, None, :, :].to_broadcast([P, NH, J, 2])

                    t_sb = tpool.tile([P, MC], F32, tag="t")
                    u_sb = tpool.tile([P, MC], F32, tag="u")
                    t_pr = t_sb[:].rearrange(
                        "p (h j two) -> p h j two", h=NH, two=2
                    )
                    u_hd = u_sb[:].rearrange("p (h d) -> p h d", d=hd)

                    nc.vector.tensor_tensor(
                        t_pr[:], ps_pr[:, :, :, ::-1], sin_b,
                        mybir.AluOpType.mult,
                    )
                    nc.vector.tensor_tensor(
                        u_hd, ps_hd, cos_b, mybir.AluOpType.mult,
                    )
                    nc.vector.tensor_tensor(
                        o_sb[:], u_sb[:], t_sb[:], mybir.AluOpType.add,
                    )
                else:
                    nc.scalar.copy(o_sb[:], ps[:])
                nc.sync.dma_start(o_r[:, rc, ds(c * MC, MC)], o_sb[:])

            def emit_phase(w_r, o_r, rope, xt_tiles, cos_sb, sin_sb):
                w_tiles = []
                for ko in range(KO):
                    for h in range(2):
                        w_sb = wpool.tile([P, WT], BF16, tag="w")
                        nc.scalar.dma_start(w_sb[:], w_r[:, ko, ts(h, WT)])
                        w_tiles.append(w_sb)

                # first two rc interleaved: halves the cold W-stream rate
                ps0 = [pspool.tile([P, MC], F32, tag="ps", name=f"ps0_{c}")
                       for c in range(NC_)]
                ps1 = [pspool.tile([P, MC], F32, tag="ps", name=f"ps1_{c}")
                       for c in range(NC_)]
                for ko in range(KO):
                    emit_group(xt_tiles, 0, ko, w_tiles, ps0,
                               ko == 0, ko == KO - 1)
                    emit_group(xt_tiles, 1, ko, w_tiles, ps1,
                               ko == 0, ko == KO - 1)
                for c in range(NC_):
                    finish_chunk(ps0[c], o_r, 0, c, rope, cos_sb, sin_sb)
                for c in range(NC_):
                    finish_chunk(ps1[c], o_r, 1, c, rope, cos_sb, sin_sb)

                for rc in range(2, RC):
                    ps = [pspool.tile([P, MC], F32, tag="ps", name=f"ps_{c}")
                          for c in range(NC_)]
                    for ko in range(KO):
                        emit_group(xt_tiles, rc, ko, w_tiles, ps,
                                   ko == 0, ko == KO - 1)
                    for c in range(NC_):
                        finish_chunk(ps[c], o_r, rc, c, rope, cos_sb, sin_sb)

            def body():
                # Cold-start ordering: the first matmuls need only x[0] and
                # the first W tiles, so issue those before everything else
                # (x on the SP HWDGE ring, W on ACT's).
                xt_tiles = []
                for rc in range(RC):
                    x_sb = xpool.tile([P, KO, P], BF16, tag="x")
                    nc.sync.dma_start(x_sb[:], xt_r[:, :, ts(rc, P)])
                    xt_tiles.append(x_sb)
                    if rc == 3:
                        cos_sb = cpool.tile([P, RC, hd], F32, tag="cos")
                        sin_sb = cpool.tile([P, RC, hd], F32, tag="sin")
                        nc.sync.dma_start(cos_sb[:], cos_r)
                        nc.sync.dma_start(sin_sb[:], sin_r)

                for w_dram, o_dram, rope in (
                    (wv, v_out, False),  # V first: no RoPE, so the cold
                    (wq, q_out, True),   # start has no cos/sin dependency
                    (wk, k_out, True),
                ):
                    w_r = w_dram[:].rearrange("(ko p) m -> p ko m", p=P)
                    o_r = o_dram[:].rearrange("(rc p) m -> p rc m", p=P)
                    emit_phase(w_r, o_r, rope, xt_tiles, cos_sb, sin_sb)

            body()

    nc.compile()
    return nc


_NC_CACHE = {}


def _get_nc():
    if "nc" not in _NC_CACHE:
        _NC_CACHE["nc"] = build_nc()
    return _NC_CACHE["nc"]


def prepare_in_maps(X, freqs_cos, freqs_sin, Wq, Wk, Wv):
    X = np.asarray(X, dtype=np.float32)
    freqs_cos = np.asarray(freqs_cos, dtype=np.float32)
    freqs_sin = np.asarray(freqs_sin, dtype=np.float32)

    Xf = X.reshape(B * S, DIM)
    Xb = Xf.astype(ml_dtypes.bfloat16)
    wq_b = np.asarray(Wq, dtype=np.float32).astype(ml_dtypes.bfloat16)
    wk_b = np.asarray(Wk, dtype=np.float32).astype(ml_dtypes.bfloat16)
    wv_b = np.asarray(Wv, dtype=np.float32).astype(ml_dtypes.bfloat16)

    # Rotation sign baked into sin: out[2i] = x[2i]c - x[2i+1]s,
    # out[2i+1] = x[2i+1]c + x[2i]s.
    ssin_full = freqs_sin.copy()
    ssin_full[:, 0::2] *= -1.0

    in_maps = []
    for c in range(N_CORES):
        rows = slice(c * R, (c + 1) * R)
        s0 = (c % 2) * R  # sequence offset of this shard (R == S // 2)
        in_maps.append({
            "xt": np.ascontiguousarray(Xb[rows].T),
            "wq": wq_b,
            "wk": wk_b,
            "wv": wv_b,
            "cosf": np.ascontiguousarray(freqs_cos[s0:s0 + R]),
            "ssin": np.ascontiguousarray(ssin_full[s0:s0 + R]),
        })
    return in_maps


def assemble_outputs(results):
    Xq = np.empty((B * S, H, HD), dtype=np.float32)
    Xk = np.empty((B * S, H, HD), dtype=np.float32)
    Xv = np.empty((B * S, H, HD), dtype=np.float32)
    for c in range(N_CORES):
        rows = slice(c * R, (c + 1) * R)
        Xq[rows] = results[c]["q"].astype(np.float32).reshape(R, H, HD)
        Xk[rows] = results[c]["k"].astype(np.float32).reshape(R, H, HD)
        Xv[rows] = results[c]["v"].astype(np.float32).reshape(R, H, HD)

    return (
        Xq.reshape(B, S, H, HD),
        Xk.reshape(B, S, H, HD),
        Xv.reshape(B, S, H, HD),
    )


def kernel(X, freqs_cos, freqs_sin, attention_mask, Wq, Wk, Wv):
    in_maps = prepare_in_maps(X, freqs_cos, freqs_sin, Wq, Wk, Wv)
    nc = _get_nc()
    res = run_bass_kernel_spmd(nc, in_maps, list(range(N_CORES)))
    return assemble_outputs(res.results)


# revision 7
# speedup vs baseline: 1.1986x; 1.1986x over previous
"""Trainium2 Bass kernel for fused QKV projection + interleaved RoPE.

Problem: X[4, 4096, 2048] @ {Wq, Wk, Wv}[2048, 2048] -> reshape to heads
[B, S, 16, 128], apply interleaved RoPE to Q and K, return (Xq, Xk, Xv).

Sharding: data-parallel over tokens. The 4*4096 = 16384 token rows are
split into 8 contiguous shards of 2048 rows (core c gets batch c//2,
sequence half c%2). Every core holds the full Wq/Wk/Wv and computes all
2048 output features for its rows; RoPE is per-token elementwise so no
communication is needed.

Device kernel (identical SPMD program on all 8 cores):
  - The PE is the bottleneck (3 x [2048,2048]@[2048,2048] bf16 per core).
    A self-loading matmul pays a ~46 ns weight-swap bubble per
    instruction (measured 259 ns for 512 moving rows vs 213 ns
    streaming). So the stationary X^T tile [128k, 128r] is loaded ONCE
    via an explicit LDWEIGHTS and reused by 4 consecutive matmuls
    (ins.ldweights = False) covering all 2048 output columns of one
    tensor in 4 x 512-column PSUM chunks.
  - Per phase (V, Q, K): for rc: 4 psum chunks [128,512] accumulate over
    ko=16 LD+4MM groups. 8 one-bank psum tiles double-buffer rc vs rc+1.
    The first two rc of each phase are interleaved (8 chunks live) so
    the cold W stream is consumed at half rate.
  - RoPE in 3 DVE ops per chunk on the psum tile: the interleaved pair
    swap is a reversed-stride access pattern, the rotation sign is
    pre-baked into the sin table on the host, and cos/sin broadcast
    across heads via zero-stride APs. V chunks are copied on the Act
    engine. Outputs are written f16 (halves DMA-out) and upcast on host.
"""

import numpy as np
import ml_dtypes

import concourse.bass as bass
import concourse.mybir as mybir
import concourse.tile as tile
from concourse import bacc
from concourse.bass import ds, ts
from concourse.bass_utils import run_bass_kernel_spmd

B, S, DIM, H = 4, 4096, 2048, 16
HD = DIM // H           # 128
N_CORES = 8
R = B * S // N_CORES    # 2048 token rows per core
P = 128

BF16 = mybir.dt.bfloat16
F16 = mybir.dt.float16
F32 = mybir.dt.float32

MC = 512                # psum chunk columns (1 PSUM bank)


def build_nc(K=DIM, M=DIM, rows=R, hd=HD):
    """Build the per-core Bass program."""
    KO = K // P           # 16 k-chunks
    RC = rows // P        # 16 token row chunks
    NC_ = M // MC         # 4 column chunks per tensor
    WT = 1024             # W sbuf tile columns
    NH = MC // hd         # heads per chunk (4)
    J = hd // 2

    nc = bacc.Bacc(None, target_bir_lowering=False)

    xt = nc.dram_tensor("xt", [K, rows], BF16, kind="ExternalInput")
    wq = nc.dram_tensor("wq", [K, M], BF16, kind="ExternalInput")
    wk = nc.dram_tensor("wk", [K, M], BF16, kind="ExternalInput")
    wv = nc.dram_tensor("wv", [K, M], BF16, kind="ExternalInput")
    cosf = nc.dram_tensor("cosf", [rows, hd], F32, kind="ExternalInput")
    ssin = nc.dram_tensor("ssin", [rows, hd], F32, kind="ExternalInput")
    q_out = nc.dram_tensor("q", [rows, M], F16, kind="ExternalOutput")
    k_out = nc.dram_tensor("k", [rows, M], F16, kind="ExternalOutput")
    v_out = nc.dram_tensor("v", [rows, M], F16, kind="ExternalOutput")

    xt_r = xt[:].rearrange("(ko p) r -> p ko r", p=P)
    cos_r = cosf[:].rearrange("(rc p) d -> p rc d", p=P)
    sin_r = ssin[:].rearrange("(rc p) d -> p rc d", p=P)

    with tile.TileContext(nc) as tc:
        with (
            tc.tile_pool(name="wpool", bufs=48) as wpool,
            tc.tile_pool(name="xpool", bufs=RC) as xpool,
            tc.tile_pool(name="cpool", bufs=1) as cpool,
            tc.tile_pool(name="opool", bufs=12) as opool,
            tc.tile_pool(name="tpool", bufs=4) as tpool,
            tc.tile_pool(name="psum", bufs=8, space="PSUM") as pspool,
        ):
            def emit_group(xt_tiles, rc, ko, w_tiles, ps_chunks, first, last):
                # NC_ matmuls sharing one stationary; the tile scheduler
                # splits each into LDWEIGHTS+MATMUL and _dedupe_ldweights
                # later drops the redundant same-stationary reloads.
                for c in range(NC_):
                    w_sb = w_tiles[2 * ko + c // 2]
                    nc.tensor.matmul(
                        ps_chunks[c][:],
                        xt_tiles[rc][:, ko],
                        w_sb[:, ts(c % 2, MC)],
                        start=first,
                        stop=last,
                    )

            def finish_chunk(ps, o_r, rc, c, rope, cos_sb, sin_sb):
                o_sb = opool.tile([P, MC], F16, tag="o")
                if rope:
                    # o = x*cos + swap_pairs(x)*ssin; ssin sign-baked,
                    # the swap is a reversed-stride AP on the pair dim.
                    ps_hd = ps[:].rearrange("p (h d) -> p h d", d=hd)
                    ps_pr = ps[:].rearrange(
                        "p (h j two) -> p h j two", h=NH, two=2
                    )
                    cos_b = cos_sb[:, rc, None, :].to_broadcast([P, NH, hd])
                    sin_b = sin_sb[:, rc].rearrange(
                        "p (j two) -> p j two", two=2
                    )[:, None, :, :].to_broadcast([P, NH, J, 2])

                    t_sb = tpool.tile([P, MC], F32, tag="t")
                    u_sb = tpool.tile([P, MC], F32, tag="u")
                    t_pr = t_sb[:].rearrange(
                        "p (h j two) -> p h j two", h=NH, two=2
                    )
                    u_hd = u_sb[:].rearrange("p (h d) -> p h d", d=hd)

                    nc.vector.tensor_tensor(
                        t_pr[:], ps_pr[:, :, :, ::-1], sin_b,
                        mybir.AluOpType.mult,
                    )
                    nc.vector.tensor_tensor(
                        u_hd, ps_hd, cos_b, mybir.AluOpType.mult,
                    )
                    nc.vector.tensor_tensor(
                        o_sb[:], u_sb[:], t_sb[:], mybir.AluOpType.add,
                    )
                else:
                    nc.scalar.copy(o_sb[:], ps[:])
                # outs own the ACT ring; x + W stream on the SP ring
                nc.scalar.dma_start(o_r[:, rc, ds(c * MC, MC)], o_sb[:])

            def emit_phase(w_tiles, o_r, rope, xt_tiles, cos_sb, sin_sb):
                # first two rc interleaved: halves the cold W-stream rate
                ps0 = [pspool.tile([P, MC], F32, tag="ps", name=f"ps0_{c}")
                       for c in range(NC_)]
                ps1 = [pspool.tile([P, MC], F32, tag="ps", name=f"ps1_{c}")
                       for c in range(NC_)]
                for ko in range(KO):
                    emit_group(xt_tiles, 0, ko, w_tiles, ps0,
                               ko == 0, ko == KO - 1)
                    emit_group(xt_tiles, 1, ko, w_tiles, ps1,
                               ko == 0, ko == KO - 1)
                for c in range(NC_):
                    finish_chunk(ps0[c], o_r, 0, c, rope, cos_sb, sin_sb)
                for c in range(NC_):
                    finish_chunk(ps1[c], o_r, 1, c, rope, cos_sb, sin_sb)

                for rc in range(2, RC):
                    ps = [pspool.tile([P, MC], F32, tag="ps", name=f"ps_{c}")
                          for c in range(NC_)]
                    for ko in range(KO):
                        emit_group(xt_tiles, rc, ko, w_tiles, ps,
                                   ko == 0, ko == KO - 1)
                    for c in range(NC_):
                        finish_chunk(ps[c], o_r, rc, c, rope, cos_sb, sin_sb)

            def body():
                # Cold start is HBM-bound: rc0/rc1 x tiles first, then the
                # phase-V W stream interleaved with the remaining x tiles,
                # all on the SP ring so arrival order matches need order.
                xt_tiles = [None] * RC

                def load_x(rc):
                    x_sb = xpool.tile([P, KO, P], BF16, tag="x",
                                      name=f"x_{rc}")
                    nc.sync.dma_start(x_sb[:], xt_r[:, :, ts(rc, P)])
                    xt_tiles[rc] = x_sb

                def load_w(w_dram, interleave_x):
                    w_r = w_dram[:].rearrange("(ko p) m -> p ko m", p=P)
                    w_tiles = []
                    for ko in range(KO):
                        for h in range(2):
                            w_sb = wpool.tile([P, WT], BF16, tag="w",
                                              name=f"w_{ko}_{h}")
                            nc.sync.dma_start(w_sb[:], w_r[:, ko, ts(h, WT)])
                            w_tiles.append(w_sb)
                        if interleave_x and ko + 2 < RC:
                            load_x(ko + 2)
                    return w_tiles

                load_x(0)
                load_x(1)
                wv_tiles = load_w(wv, True)
                cos_sb = cpool.tile([P, RC, hd], F32, tag="cos")
                sin_sb = cpool.tile([P, RC, hd], F32, tag="sin")
                nc.sync.dma_start(cos_sb[:], cos_r)
                nc.sync.dma_start(sin_sb[:], sin_r)

                for w_dram, w_tiles, o_dram, rope in (
                    (wv, wv_tiles, v_out, False),  # V first: no RoPE, so
                    (wq, None, q_out, True),       # the cold start has no
                    (wk, None, k_out, True),       # cos/sin dependency
                ):
                    if w_tiles is None:
                        w_tiles = load_w(w_dram, False)
                    o_r = o_dram[:].rearrange("(rc p) m -> p rc m", p=P)
                    emit_phase(w_tiles, o_r, rope, xt_tiles, cos_sb, sin_sb)

            body()

    _dedupe_ldweights(nc)
    nc.compile()
    return nc


def _dedupe_ldweights(nc):
    """Drop InstLdweights that reload the stationary already in the PE.

    The tile scheduler splits every matmul into LDWEIGHTS + MATMUL
    (ldweights=False). Within a run of matmuls sharing one stationary,
    only the first load is needed: walrus keeps non-self-loading matmuls
    as plain MATMULs, and skipping the redundant reloads removes the
    per-instruction weight-swap bubble on the PE. Only loads with no
    semaphore waits/updates and an access pattern identical to the
    previous load are dropped.
    """
    for f in nc.m.functions:
        for bb in f.blocks:
            last_key = None
            drop = []
            for inst in bb.instructions:
                nm = type(inst).__name__
                if nm == "InstLdweights":
                    si = inst.sync_info
                    clean = si is None or (not si.on_wait and not si.on_update)
                    key = repr(inst.ins[0])
                    if clean and key == last_key:
                        drop.append(inst)
                        continue
                    last_key = key
                elif nm != "InstMatmult":
                    if getattr(inst, "engine", None) == mybir.EngineType.PE:
                        last_key = None
            for inst in drop:
                bb.instructions.remove(inst)


_NC_CACHE = {}


def _get_nc():
    if "nc" not in _NC_CACHE:
        _NC_CACHE["nc"] = build_nc()
    return _NC_CACHE["nc"]


def prepare_in_maps(X, freqs_cos, freqs_sin, Wq, Wk, Wv):
    X = np.asarray(X, dtype=np.float32)
    freqs_cos = np.asarray(freqs_cos, dtype=np.float32)
    freqs_sin = np.asarray(freqs_sin, dtype=np.float32)

    Xf = X.reshape(B * S, DIM)
    Xb = Xf.astype(ml_dtypes.bfloat16)
    wq_b = np.asarray(Wq, dtype=np.float32).astype(ml_dtypes.bfloat16)
    wk_b = np.asarray(Wk, dtype=np.float32).astype(ml_dtypes.bfloat16)
    wv_b = np.asarray(Wv, dtype=np.float32).astype(ml_dtypes.bfloat16)

    # Rotation sign baked into sin: out[2i] = x[2i]c - x[2i+1]s,
    # out[2i+1] = x[2i+1]c + x[2i]s.
    ssin_full = freqs_sin.copy()
    ssin_full[:, 0::2] *= -1.0

    in_maps = []
    for c in range(N_CORES):
        rows = slice(c * R, (c + 1) * R)
        s0 = (c % 2) * R  # sequence offset of this shard (R == S // 2)
        in_maps.append({
            "xt": np.ascontiguousarray(Xb[rows].T),
            "wq": wq_b,
            "wk": wk_b,
            "wv": wv_b,
            "cosf": np.ascontiguousarray(freqs_cos[s0:s0 + R]),
            "ssin": np.ascontiguousarray(ssin_full[s0:s0 + R]),
        })
    return in_maps


def assemble_outputs(results):
    Xq = np.empty((B * S, H, HD), dtype=np.float32)
    Xk = np.empty((B * S, H, HD), dtype=np.float32)
    Xv = np.empty((B * S, H, HD), dtype=np.float32)
    for c in range(N_CORES):
        rows = slice(c * R, (c + 1) * R)
        Xq[rows] = results[c]["q"].astype(np.float32).reshape(R, H, HD)
        Xk[rows] = results[c]["k"].astype(np.float32).reshape(R, H, HD)
        Xv[rows] = results[c]["v"].astype(np.float32).reshape(R, H, HD)

    return (
        Xq.reshape(B, S, H, HD),
        Xk.reshape(B, S, H, HD),
        Xv.reshape(B, S, H, HD),
    )


def kernel(X, freqs_cos, freqs_sin, attention_mask, Wq, Wk, Wv):
    in_maps = prepare_in_maps(X, freqs_cos, freqs_sin, Wq, Wk, Wv)
    nc = _get_nc()
    res = run_bass_kernel_spmd(nc, in_maps, list(range(N_CORES)))
    return assemble_outputs(res.results)


# revision 10
# speedup vs baseline: 1.2611x; 1.0521x over previous
"""Trainium2 Bass kernel for fused QKV projection + interleaved RoPE.

Problem: X[4, 4096, 2048] @ {Wq, Wk, Wv}[2048, 2048] -> reshape to heads
[B, S, 16, 128], apply interleaved RoPE to Q and K, return (Xq, Xk, Xv).

Sharding: data-parallel over tokens. The 4*4096 = 16384 token rows are
split into 8 contiguous shards of 2048 rows (core c gets batch c//2,
sequence half c%2). Every core holds the full Wq/Wk/Wv and computes all
2048 output features for its rows; RoPE is per-token elementwise so no
communication is needed.

Device kernel (identical SPMD program on all 8 cores):
  - The PE is the bottleneck (3 x [2048,2048]@[2048,2048] bf16 per core).
    A self-loading matmul pays a ~46 ns weight-swap bubble per
    instruction (measured 259 ns for 512 moving rows vs 213 ns
    streaming). So the stationary X^T tile [128k, 128r] is loaded ONCE
    via an explicit LDWEIGHTS and reused by 4 consecutive matmuls
    (ins.ldweights = False) covering all 2048 output columns of one
    tensor in 4 x 512-column PSUM chunks.
  - Per phase (V, Q, K): for rc: 4 psum chunks [128,512] accumulate over
    ko=16 LD+4MM groups. 8 one-bank psum tiles double-buffer rc vs rc+1.
    The first two rc of each phase are interleaved (8 chunks live) so
    the cold W stream is consumed at half rate.
  - RoPE in 3 DVE ops per chunk on the psum tile: the interleaved pair
    swap is a reversed-stride access pattern, the rotation sign is
    pre-baked into the sin table on the host, and cos/sin broadcast
    across heads via zero-stride APs. V chunks are copied on the Act
    engine. Outputs are written f16 (halves DMA-out) and upcast on host.
"""

import numpy as np
import ml_dtypes

import concourse.bass as bass
import concourse.mybir as mybir
import concourse.tile as tile
from concourse import bacc
from concourse.bass import ds, ts
from concourse.bass_utils import run_bass_kernel_spmd

B, S, DIM, H = 4, 4096, 2048, 16
HD = DIM // H           # 128
N_CORES = 8
R = B * S // N_CORES    # 2048 token rows per core
P = 128

BF16 = mybir.dt.bfloat16
F16 = mybir.dt.float16
F32 = mybir.dt.float32

MC = 512                # psum chunk columns (1 PSUM bank)


def build_nc(K=DIM, M=DIM, rows=R, hd=HD):
    """Build the per-core Bass program."""
    KO = K // P           # 16 k-chunks
    RC = rows // P        # 16 token row chunks
    NC_ = M // MC         # 4 column chunks per tensor
    WT = 1024             # W sbuf tile columns
    NH = MC // hd         # heads per chunk (4)
    J = hd // 2

    nc = bacc.Bacc(None, target_bir_lowering=False)

    xt = nc.dram_tensor("xt", [K, rows], BF16, kind="ExternalInput")
    wq = nc.dram_tensor("wq", [K, M], BF16, kind="ExternalInput")
    wk = nc.dram_tensor("wk", [K, M], BF16, kind="ExternalInput")
    wv = nc.dram_tensor("wv", [K, M], BF16, kind="ExternalInput")
    cosf = nc.dram_tensor("cosf", [rows, hd], F32, kind="ExternalInput")
    ssin = nc.dram_tensor("ssin", [rows, hd], F32, kind="ExternalInput")
    q_out = nc.dram_tensor("q", [rows, M], F16, kind="ExternalOutput")
    k_out = nc.dram_tensor("k", [rows, M], F16, kind="ExternalOutput")
    v_out = nc.dram_tensor("v", [rows, M], F16, kind="ExternalOutput")

    xt_r = xt[:].rearrange("(ko p) r -> p ko r", p=P)
    cos_r = cosf[:].rearrange("(rc p) d -> p rc d", p=P)
    sin_r = ssin[:].rearrange("(rc p) d -> p rc d", p=P)

    with tile.TileContext(nc) as tc:
        with (
            tc.tile_pool(name="wpool", bufs=48) as wpool,
            tc.tile_pool(name="xpool", bufs=RC) as xpool,
            tc.tile_pool(name="cpool", bufs=1) as cpool,
            tc.tile_pool(name="opool", bufs=12) as opool,
            tc.tile_pool(name="tpool", bufs=4) as tpool,
            tc.tile_pool(name="psum", bufs=8, space="PSUM") as pspool,
        ):
            def emit_group(xt_tiles, rc, ko, w_tiles, ps_chunks, first, last):
                # NC_ matmuls sharing one stationary; the tile scheduler
                # splits each into LDWEIGHTS+MATMUL and _dedupe_ldweights
                # later drops the redundant same-stationary reloads.
                for c in range(NC_):
                    w_sb = w_tiles[2 * ko + c // 2]
                    nc.tensor.matmul(
                        ps_chunks[c][:],
                        xt_tiles[rc][:, ko],
                        w_sb[:, ts(c % 2, MC)],
                        start=first,
                        stop=last,
                    )

            def finish_chunk(ps, o_r, rc, c, rope, cos_sb, sin_sb):
                o_sb = opool.tile([P, MC], F16, tag="o")
                if rope:
                    # o = x*cos + swap_pairs(x)*ssin; ssin sign-baked,
                    # the swap is a reversed-stride AP on the pair dim.
                    ps_hd = ps[:].rearrange("p (h d) -> p h d", d=hd)
                    ps_pr = ps[:].rearrange(
                        "p (h j two) -> p h j two", h=NH, two=2
                    )
                    cos_b = cos_sb[:, rc, None, :].to_broadcast([P, NH, hd])
                    sin_b = sin_sb[:, rc].rearrange(
                        "p (j two) -> p j two", two=2
                    )[:, None, :, :].to_broadcast([P, NH, J, 2])

                    t_sb = tpool.tile([P, MC], F32, tag="t")
                    u_sb = tpool.tile([P, MC], F32, tag="u")
                    t_pr = t_sb[:].rearrange(
                        "p (h j two) -> p h j two", h=NH, two=2
                    )
                    u_hd = u_sb[:].rearrange("p (h d) -> p h d", d=hd)

                    nc.vector.tensor_tensor(
                        t_pr[:], ps_pr[:, :, :, ::-1], sin_b,
                        mybir.AluOpType.mult,
                    )
                    nc.vector.tensor_tensor(
                        u_hd, ps_hd, cos_b, mybir.AluOpType.mult,
                    )
                    nc.vector.tensor_tensor(
                        o_sb[:], u_sb[:], t_sb[:], mybir.AluOpType.add,
                    )
                else:
                    nc.scalar.copy(o_sb[:], ps[:])
                # outs own the ACT ring; x + W stream on the SP ring
                nc.scalar.dma_start(o_r[:, rc, ds(c * MC, MC)], o_sb[:])

            def emit_phase(w_tiles, o_r, rope, xt_tiles, cos_sb, sin_sb):
                # first two rc interleaved: halves the cold W-stream rate
                ps0 = [pspool.tile([P, MC], F32, tag="ps", name=f"ps0_{c}")
                       for c in range(NC_)]
                ps1 = [pspool.tile([P, MC], F32, tag="ps", name=f"ps1_{c}")
                       for c in range(NC_)]
                for ko in range(KO - 1):
                    emit_group(xt_tiles, 0, ko, w_tiles, ps0,
                               ko == 0, False)
                    emit_group(xt_tiles, 1, ko, w_tiles, ps1,
                               ko == 0, False)
                # split the last ko so ps0 drains while rc1 still computes
                emit_group(xt_tiles, 0, KO - 1, w_tiles, ps0, False, True)
                for c in range(NC_):
                    finish_chunk(ps0[c], o_r, 0, c, rope, cos_sb, sin_sb)
                emit_group(xt_tiles, 1, KO - 1, w_tiles, ps1, False, True)
                for c in range(NC_):
                    finish_chunk(ps1[c], o_r, 1, c, rope, cos_sb, sin_sb)

                for rc in range(2, RC):
                    ps = [pspool.tile([P, MC], F32, tag="ps", name=f"ps_{c}")
                          for c in range(NC_)]
                    for ko in range(KO):
                        emit_group(xt_tiles, rc, ko, w_tiles, ps,
                                   ko == 0, ko == KO - 1)
                    for c in range(NC_):
                        finish_chunk(ps[c], o_r, rc, c, rope, cos_sb, sin_sb)

            def body():
                # Cold start is HBM-bound: rc0/rc1 x tiles first, then the
                # phase-V W stream interleaved with the remaining x tiles,
                # all on the SP ring so arrival order matches need order.
                xt_tiles = [None] * RC

                def load_x(rc):
                    x_sb = xpool.tile([P, KO, P], BF16, tag="x",
                                      name=f"x_{rc}")
                    nc.sync.dma_start(x_sb[:], xt_r[:, :, ts(rc, P)])
                    xt_tiles[rc] = x_sb

                def load_w(w_dram):
                    w_r = w_dram[:].rearrange("(ko p) m -> p ko m", p=P)
                    w_tiles = []
                    for ko in range(KO):
                        for h in range(2):
                            w_sb = wpool.tile([P, WT], BF16, tag="w",
                                              name=f"w_{ko}_{h}")
                            nc.sync.dma_start(w_sb[:], w_r[:, ko, ts(h, WT)])
                            w_tiles.append(w_sb)
                    return w_tiles

                # rc0/rc1 x tiles, then the whole wv stream (pair0 needs it
                # by ~30us), then the remaining x tiles (rc2 isn't needed
                # until pair0 completes), then cos/sin (Q phase only).
                load_x(0)
                load_x(1)
                wv_tiles = load_w(wv)
                for rc in range(2, RC):
                    load_x(rc)
                cos_sb = cpool.tile([P, RC, hd], F32, tag="cos")
                sin_sb = cpool.tile([P, RC, hd], F32, tag="sin")
                nc.sync.dma_start(cos_sb[:], cos_r)
                nc.sync.dma_start(sin_sb[:], sin_r)

                for w_dram, w_tiles, o_dram, rope in (
                    (wv, wv_tiles, v_out, False),  # V first: no RoPE, so
                    (wq, None, q_out, True),       # the cold start has no
                    (wk, None, k_out, True),       # cos/sin dependency
                ):
                    if w_tiles is None:
                        w_tiles = load_w(w_dram)
                    o_r = o_dram[:].rearrange("(rc p) m -> p rc m", p=P)
                    emit_phase(w_tiles, o_r, rope, xt_tiles, cos_sb, sin_sb)

            body()

    _dedupe_ldweights(nc)
    nc.compile()
    return nc


def _dedupe_ldweights(nc):
    """Drop InstLdweights that reload the stationary already in the PE.

    The tile scheduler splits every matmul into LDWEIGHTS + MATMUL
    (ldweights=False). Within a run of matmuls sharing one stationary,
    only the first load is needed: walrus keeps non-self-loading matmuls
    as plain MATMULs, and skipping the redundant reloads removes the
    per-instruction weight-swap bubble on the PE. Only loads with no
    semaphore waits/updates and an access pattern identical to the
    previous load are dropped.
    """
    for f in nc.m.functions:
        for bb in f.blocks:
            last_key = None
            drop = []
            for inst in bb.instructions:
                nm = type(inst).__name__
                if nm == "InstLdweights":
                    si = inst.sync_info
                    clean = si is None or (not si.on_wait and not si.on_update)
                    key = repr(inst.ins[0])
                    if clean and key == last_key:
                        drop.append(inst)
                        continue
                    last_key = key
                elif nm != "InstMatmult":
                    if getattr(inst, "engine", None) == mybir.EngineType.PE:
                        last_key = None
            for inst in drop:
                bb.instructions.remove(inst)


_NC_CACHE = {}


def _get_nc():
    if "nc" not in _NC_CACHE:
        _NC_CACHE["nc"] = build_nc()
    return _NC_CACHE["nc"]


def prepare_in_maps(X, freqs_cos, freqs_sin, Wq, Wk, Wv):
    X = np.asarray(X, dtype=np.float32)
    freqs_cos = np.asarray(freqs_cos, dtype=np.float32)
    freqs_sin = np.asarray(freqs_sin, dtype=np.float32)

    Xf = X.reshape(B * S, DIM)
    Xb = Xf.astype(ml_dtypes.bfloat16)
    wq_b = np.asarray(Wq, dtype=np.float32).astype(ml_dtypes.bfloat16)
    wk_b = np.asarray(Wk, dtype=np.float32).astype(ml_dtypes.bfloat16)
    wv_b = np.asarray(Wv, dtype=np.float32).astype(ml_dtypes.bfloat16)

    # Rotation sign baked into sin: out[2i] = x[2i]c - x[2i+1]s,
    # out[2i+1] = x[2i+1]c + x[2i]s.
    ssin_full = freqs_sin.copy()
    ssin_full[:, 0::2] *= -1.0

    in_maps = []
    for c in range(N_CORES):
        rows = slice(c * R, (c + 1) * R)
        s0 = (c % 2) * R  # sequence offset of this shard (R == S // 2)
        in_maps.append({
            "xt": np.ascontiguousarray(Xb[rows].T),
            "wq": wq_b,
            "wk": wk_b,
            "wv": wv_b,
            "cosf": np.ascontiguousarray(freqs_cos[s0:s0 + R]),
            "ssin": np.ascontiguousarray(ssin_full[s0:s0 + R]),
        })
    return in_maps


def assemble_outputs(results):
    Xq = np.empty((B * S, H, HD), dtype=np.float32)
    Xk = np.empty((B * S, H, HD), dtype=np.float32)
    Xv = np.empty((B * S, H, HD), dtype=np.float32)
    for c in range(N_CORES):
        rows = slice(c * R, (c + 1) * R)
        Xq[rows] = results[c]["q"].astype(np.float32).reshape(R, H, HD)
        Xk[rows] = results[c]["k"].astype(np.float32).reshape(R, H, HD)
        Xv[rows] = results[c]["v"].astype(np.float32).reshape(R, H, HD)

    return (
        Xq.reshape(B, S, H, HD),
        Xk.reshape(B, S, H, HD),
        Xv.reshape(B, S, H, HD),
    )


def kernel(X, freqs_cos, freqs_sin, attention_mask, Wq, Wk, Wv):
    in_maps = prepare_in_maps(X, freqs_cos, freqs_sin, Wq, Wk, Wv)
    nc = _get_nc()
    res = run_bass_kernel_spmd(nc, in_maps, list(range(N_CORES)))
    return assemble_outputs(res.results)
